# revision 27
# baseline (speedup 1.0000x reference)
"""Trainium2 Bass kernel for additive-attention pooling (sparse_attention).

Reference computation (per batch b):
    pv   = values[b] @ W_in                  # [T, A]
    pq   = query[b] @ W_q                    # [A]
    s    = tanh(pv + pq) @ v_w + v_b         # [T, 1]
    attn = sigmoid(s); attn /= sum(attn)
    out  = attn.T @ values[b]                # [1, D]

Shapes: B=16, T=8192, D=512, A=128. Memory-bound: the only large tensor is
`values` (256 MB fp32).

Strategy: data-parallel over batch, 2 batches per NeuronCore on 8 cores.
Each core streams its 32 MB `values` shard from HBM exactly once (SWDGE
cast-DMA fp32->bf16, 2 MB per transfer with 16 KB-contiguous runs per
partition), and both the score pass and the weighted accumulation consume
the same SBUF-resident chunk:

  - The D-contraction (values @ W_in) needs `values` with D on partitions;
    that transpose is done on the PE as a *regular* matmul against a bf16
    identity (fast-weight-load + warm clock; ~56 ns per 128x128 tile).
  - ACT applies tanh(.+pq) (per-partition bias) on the pv^T tile; PE
    reduces over A against v_w; ACT applies sigmoid(.+v_b) and emits the
    per-chunk attention sum as the activation's accum_out; PE accumulates
    ws += attn^T @ values into one PSUM bank across the whole batch; a
    single normalization by sum(attn) happens at the end.

The emission is software-pipelined so the PE never sits behind the ACT
chain: at pipeline step i the PE runs transposes+pv of chunk i, the score
matmuls of chunk i-1 (tanh of i-1 ran during chunk i's transposes), and
the weighted-sum matmuls of chunk i-2 (sigmoid of i-2 ran during step
i-1).  PSUM->SBUF cast copies of the transposed tiles rotate over
DVE/ACT/GpSimd so no single engine sits on the critical path.  All small
params are pre-cast to bf16 on the host and loaded via HWDGE (sync) so the
SWDGE queues carry nothing but the values stream.

The within-128-tile t ordering is interleaved (t = base + p*8 + n) so each
SBUF partition reads one contiguous 16 KB run per DMA; every t-indexed
tensor (scores, attn, the mm2 reduction) uses the same mapping, and all
T-reductions are order-independent, so the result is unchanged.

The tiny projections query@W_q and the v_b broadcast are precomputed on
the host (~1 MFLOP, irrelevant next to the 17 GFLOP / 256 MB main pass).
"""

import os
import numpy as np
import ml_dtypes

import concourse.bacc as bacc
import concourse.mybir as mybir
import concourse.tile as tile
from concourse.bass_utils import run_bass_kernel_spmd

F32 = mybir.dt.float32
BF16 = mybir.dt.bfloat16

B, T, D, A = 16, 8192, 512, 128
N_CORES = 8
B_PER_CORE = B // N_CORES          # 2
CT = 512                           # t-rows per compute chunk
NCHUNK = T // CT                   # 16 per batch
NCHUNK_ALL = NCHUNK * B_PER_CORE   # 32 chunk pipeline steps per core
NT = CT // 128                     # 4 t-tiles per chunk
NC_D = D // 128                    # 4 d-chunks
DMA_CHUNKS = 4                     # compute chunks per DMA (4 MB reads)
NTB = NT * DMA_CHUNKS              # t-tiles per DMA buffer
N_WARM = 24                        # junk matmuls to pre-warm the PE clock

LAST_EXEC_TIME_NS = None
_CACHE = {}


def _build():
    nc = bacc.Bacc("TRN2", target_bir_lowering=False, debug=False,
                   num_devices=N_CORES, num_swdge_queues=2)

    values = nc.dram_tensor("values", [B_PER_CORE, T, D], F32, kind="ExternalInput")
    # all bf16 consts packed in one tensor (one HWDGE transfer):
    # [ident(128) | W_in d-major (4*128) | v_w(1)] = [128, 641]
    cb = nc.dram_tensor("cb", [128, 128 + NC_D * A + 1], BF16, kind="ExternalInput")
    # f32 consts (gate only ACT / the tail): [pq(2) | vb(1) | ones(1)]
    cf = nc.dram_tensor("cf", [128, B_PER_CORE + 2], F32, kind="ExternalInput")
    ctx_out = nc.dram_tensor("ctx", [B_PER_CORE, D], F32, kind="ExternalOutput")

    with tile.TileContext(nc) as tc:
        with (
            tc.tile_pool(name="const", bufs=1) as consts,
            tc.tile_pool(name="vnat", bufs=4) as p_nat,
            tc.tile_pool(name="vt", bufs=6) as p_vt,
            tc.tile_pool(name="th", bufs=3) as p_th,
            tc.tile_pool(name="stats", bufs=2) as p_stats,
            tc.tile_pool(name="outs", bufs=2) as p_out,
            tc.tile_pool(name="ps_tr", bufs=4, space="PSUM") as ps_tr,
            tc.tile_pool(name="ps_pv", bufs=2, space="PSUM") as ps_pv,
            tc.tile_pool(name="ps_small", bufs=1, space="PSUM") as ps_small,
            tc.tile_pool(name="ps_ws", bufs=1, space="PSUM") as ps_ws,
        ):
            # --- values transfer plan: (global_chunk, n_chunks) per buffer.
            # Chunk 0 is split into four 256 KB t-tile transfers so the PE
            # can start on the first quarter; the rest ramp up to 4 MB
            # transfers (64 KB contiguous per partition).  All hoisted
            # before the const loads so the SWDGE queue starts immediately.
            plan = [(0, 1), (1, 1), (2, 2)] + \
                [(4 + 4 * k, 4) for k in range((NCHUNK_ALL - 4) // 4)]

            def start_vdma(g0, nch, split=False):
                b, c0 = g0 // NCHUNK, g0 % NCHUNK
                v_nat = p_nat.tile([128, NTB, D], BF16, tag="vnat")
                if split:
                    for j in range(NT):
                        nc.gpsimd.dma_start(
                            v_nat[:, j:j + 1, :],
                            values[b, c0 * CT + j * 128:c0 * CT + (j + 1) * 128, :]
                            .rearrange("(p n) d -> p n d", p=128),
                        )
                else:
                    nc.gpsimd.dma_start(
                        v_nat[:, 0:NT * nch, :],
                        values[b, c0 * CT:(c0 + nch) * CT, :]
                        .rearrange("(p n) d -> p n d", p=128),
                    )
                return v_nat

            vbuf = {}      # global chunk -> (v_nat tile, slot offset)
            g0, nch = plan[0]
            v0 = start_vdma(g0, nch, split=True)
            for h in range(nch):
                vbuf[g0 + h] = (v0, h * NT)

            # --- constants / small params: two HWDGE transfers total (these
            # cost ~5us EACH under SWDGE load, so packing matters)
            cb_sb = consts.tile([128, 128 + NC_D * A + 1], BF16)
            nc.sync.dma_start(cb_sb[:], cb[:])
            cf_sb = consts.tile([128, B_PER_CORE + 2], F32)
            nc.sync.dma_start(cf_sb[:], cf[:])
            id_sb = cb_sb[:, 0:128]
            w_c = [cb_sb[:, 128 + c * A:128 + (c + 1) * A] for c in range(NC_D)]
            vw_sb = cb_sb[:, 128 + NC_D * A:128 + NC_D * A + 1]
            pq_sb = cf_sb[:, 0:B_PER_CORE]
            vb_sb = cf_sb[:, B_PER_CORE:B_PER_CORE + 1]
            ones_sb = cf_sb[:, B_PER_CORE + 1:B_PER_CORE + 2]

            # Pre-warm the PE's HAM clock gate with junk matmuls (gated only
            # on the on-chip ident): the first real matmul then runs closer
            # to 2.4 GHz instead of paying the ~3.4us half-clock ramp.
            warm_ps = ps_small.tile([128, 128], F32, name="warm", tag="small")
            for k in range(N_WARM):
                nc.tensor.matmul(warm_ps[:], id_sb, id_sb,
                                 start=True, stop=True, skip_group_check=True)

            # Zeroed [128, 128] stationary slabs for the weighted-sum
            # matmuls: column 0 carries attn, the rest stay zero.  A full
            # 128-wide stationary gets fast-weight-load and pipelines, where
            # a 1-column stationary costs ~+90ns per matmul.
            attnw = []
            for k in range(3):
                aw = consts.tile([128, NT, 128], BF16, name=f"attnw{k}")
                nc.vector.memset(aw[:], 0)
                attnw.append(aw)

            for g0, nch in plan[1:]:
                v = start_vdma(g0, nch)
                for h in range(nch):
                    vbuf[g0 + h] = (v, h * NT)

            # PSUM->SBUF cast copies: GPSIMD can't read PSUM on TRN2, so
            # split between DVE and ACT, sized so both engines end up at
            # ~59 us total (ACT also runs tanh+sigmoid).
            def copy_engine(ci, c):
                if c == 1 or (c == 3 and ci % 8 < 3):
                    return nc.scalar.copy
                return nc.vector.tensor_copy

            # per-chunk pipeline state
            th_t = {}      # g -> th tile
            asum = {}      # batch -> [128, NCHUNK] chunk attn sums
            ws = {}        # batch -> [128, D] psum accumulator (row 0 real)
            rinv_t = {}    # batch -> reciprocal attn sum
            pvq = []       # pending pv-stage closures (2-stage lag so the
                           # PSUM->SBUF cast of the transpose lands first)
            PV_LAG = 2

            def pop_pv():
                if pvq:
                    pvq.pop(0)()

            for i in range(NCHUNK_ALL + 2):
                # chunk g=i: transposes (+lagged pv/tanh); chunk g-1: score+
                # sigmoid; chunk g-2: weighted sum.  pv/ws matmuls are
                # interleaved between transpose groups so their PSUM drains
                # hide under the next group's fill.
                if i < NCHUNK_ALL:
                    g = i
                    b, ci = g // NCHUNK, g % NCHUNK
                    if ci == 0:
                        ws[b] = ps_ws.tile([128, D], F32, name="ws", tag="ws")
                        asum[b] = p_stats.tile([128, NCHUNK], F32, name="asum", tag="asum")
                    v_nat, j0 = vbuf[g]
                    vt = p_vt.tile([128, NC_D, CT], BF16, tag="vt")
                    pv_ps = ps_pv.tile([A, CT], F32, name="pv_ps", tag="pv")

                    def mk_pv(pv_ps, vt, w, c, g, b):
                        def emit():
                            nc.tensor.matmul(
                                pv_ps[:], w, vt[:, c, :],
                                start=(c == 0), stop=(c == NC_D - 1),
                                skip_group_check=True,
                            )
                            if c == NC_D - 1:
                                th = p_th.tile([A, CT], BF16, tag="th")
                                nc.scalar.activation(
                                    th[:], pv_ps[:],
                                    mybir.ActivationFunctionType.Tanh,
                                    bias=pq_sb[:, b:b + 1],
                                )
                                th_t[g] = th
                        return emit
                else:
                    g = None

                g2 = i - 2
                if g2 >= 0:
                    b2, ci2 = g2 // NCHUNK, g2 % NCHUNK
                    attn2 = attnw[g2 % 3]
                    v_nat2, j02 = vbuf.pop(g2)

                for c in range(NC_D):
                    if g is not None:
                        # transpose as REGULAR matmul: tr = v_tile^T @ I.
                        # chunk 0 runs j-major so each transpose waits only
                        # on its own 256 KB quarter-transfer.
                        if g == 0:
                            if c == 0:
                                tr0 = [ps_tr.tile([128, CT], F32,
                                                  name=f"tr0_{cc}", tag="tr")
                                       for cc in range(NC_D)]
                            for cc in range(NC_D):
                                nc.tensor.matmul(
                                    tr0[cc][:, c * 128:(c + 1) * 128],
                                    v_nat[:, j0 + c, cc * 128:(cc + 1) * 128],
                                    id_sb,
                                    start=True, stop=True,
                                    skip_group_check=True,
                                )
                            if c == NC_D - 1:
                                for cc in range(NC_D):
                                    copy_engine(ci, cc)(vt[:, cc, :], tr0[cc][:])
                                    pvq.append(mk_pv(pv_ps, vt, w_c[cc], cc, g, b))
                        else:
                            tr_ps = ps_tr.tile([128, CT], F32, name="tr_ps", tag="tr")
                            for j in range(NT):
                                nc.tensor.matmul(
                                    tr_ps[:, j * 128:(j + 1) * 128],
                                    v_nat[:, j0 + j, c * 128:(c + 1) * 128],
                                    id_sb,
                                    start=True, stop=True,
                                    skip_group_check=True,
                                )
                            copy_engine(ci, c)(vt[:, c, :], tr_ps[:])
                            pvq.append(mk_pv(pv_ps, vt, w_c[c], c, g, b))
                            if len(pvq) > PV_LAG:
                                pop_pv()
                    elif pvq:
                        pop_pv()
                    if g2 >= 0:
                        # ws[0, d] += sum_t attn[t] * values[t, d], t-tile c
                        # (stationary is the 128-wide slab; rows 1..127 of
                        # the psum accumulate zeros)
                        nc.tensor.matmul(
                            ws[b2][:], attn2[:, c, :], v_nat2[:, j02 + c, :],
                            start=(ci2 == 0 and c == 0),
                            stop=(ci2 == NCHUNK - 1 and c == NT - 1),
                            skip_group_check=True,
                        )

                # ---- score matmuls + sigmoid for chunk g-1
                g1 = i - 1
                if 0 <= g1 < NCHUNK_ALL:
                    b1, ci1 = g1 // NCHUNK, g1 % NCHUNK
                    while g1 not in th_t and pvq:
                        pop_pv()
                    assert g1 in th_t
                    th1 = th_t.pop(g1)
                    sc_ps = ps_small.tile([128, NT], F32, tag="small")
                    for j in range(NT):
                        nc.tensor.matmul(
                            sc_ps[:, j:j + 1],
                            th1[:, j * 128:(j + 1) * 128], vw_sb,
                            start=True, stop=True,
                            skip_group_check=True,
                        )
                    nc.scalar.activation(
                        attnw[g1 % 3][:, :, 0], sc_ps[:],
                        mybir.ActivationFunctionType.Sigmoid,
                        bias=vb_sb,
                        accum_out=asum[b1][:, ci1:ci1 + 1],
                    )
                    # hoist the attn-sum reduction: everything except the
                    # final scale+store runs before the last ws matmuls
                    if ci1 == NCHUNK - 1:
                        ssum = p_stats.tile([128, 1], F32, tag="ssum")
                        nc.vector.reduce_sum(ssum[:], asum[b1][:],
                                             axis=mybir.AxisListType.X)
                        s_ps = ps_small.tile([1, 1], F32, tag="small")
                        nc.tensor.matmul(s_ps[:], ssum[:], ones_sb,
                                         start=True, stop=True,
                                         skip_group_check=True)
                        rinv = p_stats.tile([1, 1], F32, tag="rinv")
                        nc.vector.reciprocal(rinv[:], s_ps[:])
                        rinv_t[b1] = rinv

                # ---- batch tail: ctx = ws[0, :] / sum(attn)
                if g2 >= 0 and ci2 == NCHUNK - 1:
                    ctx_sb = p_out.tile([1, D], F32, tag="ctx")
                    nc.vector.tensor_scalar_mul(ctx_sb[:], ws[b2][0:1, :],
                                                rinv_t.pop(b2)[:])
                    nc.sync.dma_start(ctx_out[b2:b2 + 1, :], ctx_sb[:])

    nc.compile()
    return nc


def _enable_axon_ntff_tracing():
    """Dev-only (KERNEL_TRACE=1): register the NTFF profile hook that the
    agent image's antenv package is missing, and keep profile artifacts
    local instead of uploading."""
    import sys
    import types

    if "antenv.axon_hooks" not in sys.modules:
        mod = types.ModuleType("antenv.axon_hooks")
        mod._hook = None
        mod.set_axon_ntff_profile_hook = lambda h: setattr(mod, "_hook", h)
        mod.get_axon_ntff_profile_hook = lambda: mod._hook
        sys.modules["antenv.axon_hooks"] = mod
        from trn_agent_boot.trn_boot import _ntff_profile_via_ctypes
        mod.set_axon_ntff_profile_hook(
            _ntff_profile_via_ctypes("/opt/axon/libaxon_pjrt.so"))

    import concourse.bass_utils as bu
    bu.upload_artifacts = lambda tmpdir: tmpdir


def kernel(query, values, W_in, W_q, v_w, v_b):
    global LAST_EXEC_TIME_NS
    query = np.asarray(query, dtype=np.float32)
    values = np.asarray(values, dtype=np.float32)
    W_in = np.asarray(W_in, dtype=np.float32)
    W_q = np.asarray(W_q, dtype=np.float32)
    v_w = np.asarray(v_w, dtype=np.float32)
    v_b = np.asarray(v_b, dtype=np.float32)

    if "nc" not in _CACHE:
        _CACHE["nc"] = _build()
    nc = _CACHE["nc"]

    pq = query @ W_q                                   # [B, A] on host (tiny)
    # bf16 const block: [ident(128) | W_in d-major (c,a) | v_w]
    w_host = W_in.reshape(NC_D, 128, A).transpose(1, 0, 2).reshape(128, NC_D * A)
    cb_host = np.ascontiguousarray(np.concatenate(
        [np.eye(128, dtype=np.float32), w_host, v_w.reshape(A, 1)], axis=1,
    )).astype(ml_dtypes.bfloat16)
    in_maps = []
    for k in range(N_CORES):
        sl = slice(k * B_PER_CORE, (k + 1) * B_PER_CORE)
        # f32 const block: [pq^T (2 cols) | vb | ones]
        cf_host = np.concatenate([
            pq[sl].T.astype(np.float32),
            np.full((128, 1), float(v_b[0]), dtype=np.float32),
            np.ones((128, 1), dtype=np.float32),
        ], axis=1)
        in_maps.append({
            "values": np.ascontiguousarray(values[sl]),
            "cb": cb_host,
            "cf": np.ascontiguousarray(cf_host),
        })

    trace = bool(int(os.environ.get("KERNEL_TRACE", "0")))
    if trace:
        _enable_axon_ntff_tracing()
    res = run_bass_kernel_spmd(nc, in_maps, core_ids=list(range(N_CORES)),
                               trace=trace,
                               tmpdir=os.environ.get("KERNEL_TRACE_DIR"))
    LAST_EXEC_TIME_NS = res.exec_time_ns
    out = np.concatenate([r["ctx"] for r in res.results], axis=0)  # [B, D]
    return out.reshape(B, 1, D).astype(np.float32)
